# revision 14
# baseline (speedup 1.0000x reference)
"""Trainium2 Bass kernel for nn_Eq1to2 (segment_reduce / equivariant 1->2 layer).

Math (derived from the reference):
  out[n,i,j,s] = leaky_relu( A[n,i,s] + B[n,j,s] + G[n,s]
                             + (i==j) * (D[n,i,s] + Gd[n,s]) ) * mask
with
  A  = x @ W3                       (col term, i-dependent)
  B  = x @ W2                       (row term, j-dependent)
  D  = x @ W1                       (extra diagonal term)
  G  = sum_a agg_a @ W5_a + bias    (per-sample constant)
  Gd = sum_a agg_a @ W4_a           (per-sample diagonal constant)
where the 20 basis ops collapse to W1..W3 = sums of 4 coef slices each and
per-aggregation W4_a / W5_a; agg_a in {sum/49, sum/nobj, max, min} over N.

Sharding: pure data parallel, 1 batch sample per NeuronCore (B=8, 8 cores).

Device strategy per core (output tile [i=128 part, (j,s)=8192 free] fp32):
  - per 512-col chunk, 2 accumulating fp16 matmuls (fp16 rhs; fp16 hi/lo
    split of lhsT, error ~2^-11 of the rhs rounding only) with
    lhsT=[xT; ones] (K=65) and
    rhs=[W3 tiled 128x along j; flat(B + G + bias)] compute A + B + G + bias
    in PSUM (TensorE does both the partition and free broadcasts).
  - W3rep is materialized by SBUF->SBUF DMA with a stride-0 broadcast source.
  - ACT evicts PSUM->SBUF, DVE computes leaky via one fused
    scalar_tensor_tensor: out = (z * 0.01) max z.
  - bulk DMA the [128, 8192] tile to DRAM, then one small strided DMA
    overwrites the 128 diagonal (i==j) rows with the corrected
    leaky(A+B+D+G+Gd+bias) values computed exactly in fp32 (diagonal of
    [N,N,S] is a regular stride-(N+1)*S pattern in linear DRAM; HWDGE DMAs
    are FIFO per engine, plus explicit scheduler deps).
"""

import numpy as np

B, N, C, S = 8, 128, 64, 64
AVG_NOBJ = np.float32(49.0)
NEG = 0.01

# fp32 packed input column layout
_LHS0 = 0        # lhsT [65, 128]: rows 0:64 xT, row 64 ones (fp32, small mms)
_RHSS = 128      # rhs_small [65, 128]: cols 0:64 diag W, 64:128 B' W
_AGG0 = 256      # 3 x [64, 128] agg rhs (sum-combined, max, min)
_ONES0 = 640     # ones row at partition 0 [1, 128] (K=1 matmul lhsT)
_INF = 768       # total packed fp32 cols

# fp16 packed input column layout
_BLH = 0         # lhsT_hi [65, 128] (row 64 ones)
_BLL = 128       # lhsT_lo [65, 128] (row 64 zero)
_BW3 = 256       # W3 (fp16) tiled x16 [64, 1024]
_BNF = 1280

_CACHE = {}


def _build_nc():
    import concourse.bacc as bacc
    import concourse.bass as bass
    import concourse.mybir as mybir
    from concourse.tile import TileContext
    from concourse.tile_rust import add_dep_helper

    F32 = mybir.dt.float32
    FP16 = mybir.dt.float16
    Alu = mybir.AluOpType

    nc = bacc.Bacc("TRN2", debug=False, num_devices=8)
    inp_d = nc.dram_tensor("inp", [128, _INF], F32, kind="ExternalInput")
    inpb_d = nc.dram_tensor("inpb", [128, _BNF], FP16, kind="ExternalInput")
    out_d = nc.dram_tensor("out", [128, N * S], F32, kind="ExternalOutput")

    NB = 8           # big chunks
    CW = 1024        # chunk width (free elems)

    with TileContext(nc) as tc:
        with tc.tile_pool(name="main", bufs=1) as pool, \
             tc.tile_pool(name="tz", bufs=3) as tzpool, \
             tc.tile_pool(name="pz", bufs=3, space="PSUM") as pzpool, \
             tc.tile_pool(name="psm", bufs=1, space="PSUM") as psmpool:

            inp = pool.tile([128, _INF], F32)
            inpb = pool.tile([128, _BNF], FP16)
            rhs_t = []
            for c in range(NB):
                rhs_c = pool.tile([65, 1024], FP16, tag=f"rhs{c}")
                rhs_t.append(rhs_c)
            outbuf = pool.tile([128, N * S], F32)
            aggs = pool.tile([64, 4], F32)
            ggrow = pool.tile([1, 64], F32)
            g16f = pool.tile([1, 64], F32)
            bp_hi = pool.tile([128, 64], FP16)
            dz = pool.tile([128, 64], F32)
            dleaky = pool.tile([128, 64], F32)

            nc.sync.dma_start(out=inp[:, :], in_=inp_d[:, :])
            nc.sync.dma_start(out=inpb[:, :], in_=inpb_d[:, :])

            lhsT = inp[0:65, _LHS0:_LHS0 + 128]
            ones_p0 = inp[0:1, _ONES0:_ONES0 + 128]
            xT = inp[0:64, _LHS0:_LHS0 + 128]
            lh_hi = inpb[0:65, _BLH:_BLH + 128]
            lh_lo = inpb[0:65, _BLL:_BLL + 128]
            w3h16 = inpb[0:64, _BW3:_BW3 + 1024]

            # W3rep rows 0:64 per chunk tile: copy the host-pretiled block
            # (per-chunk tiles keep each matmul's writer set minimal)
            for c in range(NB):
                nc.gpsimd.dma_start(out=rhs_t[c][0:64, :], in_=w3h16)

            # aggregations over N (free dim of xT)
            nc.vector.tensor_reduce(out=aggs[:, 0:1], in_=xT,
                                    axis=mybir.AxisListType.X, op=Alu.add)
            nc.vector.tensor_reduce(out=aggs[:, 1:2], in_=xT,
                                    axis=mybir.AxisListType.X, op=Alu.max)
            nc.vector.tensor_reduce(out=aggs[:, 2:3], in_=xT,
                                    axis=mybir.AxisListType.X, op=Alu.min)

            # B' matmul: B' = x @ W2 + bias (ones row), then += G via a
            # K=1 matmul, so row 64 of rhs carries the full B+G+bias
            psum_sm = psmpool.tile([128, 128], F32)
            psum_bp = psum_sm[:, 64:128]
            psum_diag = psum_sm[:, 0:64]
            nc.tensor.matmul(psum_bp, lhsT,
                             inp[0:65, _RHSS + 64:_RHSS + 128],
                             start=True, stop=False)

            # [Gd + G | G] row via 3 accumulating M=1 matmuls
            # (host packs W4+W5 into the diag agg cols)
            psum_gg = psmpool.tile([1, 128], F32)
            for a in range(3):
                nc.tensor.matmul(psum_gg[0:1, :], aggs[:, a:a + 1],
                                 inp[0:64, _AGG0 + 128 * a:_AGG0 + 128 * (a + 1)],
                                 start=(a == 0), stop=(a == 2))
            nc.scalar.copy(ggrow[:, :], psum_gg[0:1, 0:64])
            nc.vector.tensor_copy(g16f[:, :], psum_gg[0:1, 64:128])
            nc.tensor.matmul(psum_bp, ones_p0, g16f[0:1, :],
                             start=False, stop=True)
            nc.scalar.copy(bp_hi[:, :], psum_bp)
            # per-chunk flatten of the BG row into each rhs tile's row 64
            for c in range(NB):
                nc.sync.dma_start(out=rhs_t[c][64:65, :],
                                  in_=bp_hi[16 * c:16 * (c + 1), :])

            # diag matmul: diag_z = x @ (W1+W2+W3) + bias + (Gd+G)
            nc.tensor.matmul(psum_diag, lhsT, inp[0:65, _RHSS:_RHSS + 64],
                             start=True, stop=False)
            nc.tensor.matmul(psum_diag, ones_p0, ggrow[0:1, :],
                             start=False, stop=True)

            # diag path (exact fp32): leaky(A+B+D+G+Gd+bias)
            nc.scalar.copy(dz[:, :], psum_diag)
            nc.vector.scalar_tensor_tensor(out=dleaky[:, :], in0=dz[:, :],
                                           scalar=NEG, in1=dz[:, :],
                                           op0=Alu.mult, op1=Alu.max)

            # big chunks: 2x fp16 matmul (hh+lh) -> ACT evict -> DVE fused
            # leaky -> outbuf -> per-chunk bulk DMA -> per-chunk diagonal
            # overwrite on the same HWDGE ring (FIFO per engine)
            flat = out_d[:, :].rearrange("a b -> (a b)")
            for c in range(NB):
                pz = pzpool.tile([128, CW], F32)
                for h2 in range(CW // 512):
                    o = pz[:, h2 * 512:(h2 + 1) * 512]
                    r = rhs_t[c][0:65, h2 * 512:(h2 + 1) * 512]
                    nc.tensor.matmul(o, lh_hi, r, start=True, stop=False)
                    nc.tensor.matmul(o, lh_lo, r, start=False, stop=True)
                last = (c == NB - 1)
                for h in range(2 if last else 1):
                    w = CW // 2 if last else CW
                    sl = slice(c * CW + h * w, c * CW + (h + 1) * w)
                    t = tzpool.tile([128, CW], F32, tag="t")
                    nc.scalar.copy(t[:, 0:w], pz[:, h * w:(h + 1) * w])
                    nc.vector.scalar_tensor_tensor(
                        out=outbuf[:, sl], in0=t[:, 0:w], scalar=NEG,
                        in1=t[:, 0:w], op0=Alu.mult, op1=Alu.max)
                    eng = nc.sync if c % 2 == 0 else nc.scalar
                    bulk = eng.dma_start(out=out_d[:, sl], in_=outbuf[:, sl])
                    if not last or h == 1:
                        # diagonal rows i in [16c, 16c+16) live in chunk c
                        dap = bass.AP(flat.tensor,
                                      flat.offset + 16 * c * (N + 1) * S,
                                      [[(N + 1) * S, 16], [1, S]])
                        ddma = eng.dma_start(out=dap,
                                             in_=dleaky[16 * c:16 * (c + 1), :])
                        add_dep_helper(ddma.ins, bulk.ins,
                                       reason="diag overwrite after bulk")

    nc.compile()
    return nc


def _get_nc():
    if "nc" not in _CACHE:
        _CACHE["nc"] = _build_nc()
    return _CACHE["nc"]


def _host_pack(inputs, nobj, coefs, bias):
    import ml_dtypes

    x = np.asarray(inputs, np.float32)        # [B, N, C]
    nobj = np.asarray(nobj, np.float32)       # [B]
    c = np.asarray(coefs, np.float32)         # [C, S, 20]
    bias = np.asarray(bias, np.float32)       # [S]

    W1 = c[:, :, 0] + c[:, :, 5] + c[:, :, 10] + c[:, :, 15]
    W2 = c[:, :, 1] + c[:, :, 6] + c[:, :, 11] + c[:, :, 16]
    W3 = c[:, :, 2] + c[:, :, 7] + c[:, :, 12] + c[:, :, 17]
    W4 = [c[:, :, 3 + 5 * a] for a in range(4)]   # sum, mean, max, min
    W5 = [c[:, :, 4 + 5 * a] for a in range(4)]

    f16 = np.float16
    W3_16t = np.tile(W3.astype(f16), (1, 16))

    in_maps = []
    for n in range(B):
        inp = np.zeros((128, _INF), np.float32)
        inp[0:64, _LHS0:_LHS0 + 128] = x[n].T
        inp[64, _LHS0:_LHS0 + 128] = 1.0
        inp[0:64, _RHSS:_RHSS + 64] = W1 + W2 + W3
        inp[64, _RHSS:_RHSS + 64] = bias
        inp[0:64, _RHSS + 64:_RHSS + 128] = W2
        inp[64, _RHSS + 64:_RHSS + 128] = bias
        W4sm = W4[0] / AVG_NOBJ + W4[1] / nobj[n]
        W5sm = W5[0] / AVG_NOBJ + W5[1] / nobj[n]
        pairs = [(W4sm, W5sm), (W4[2], W5[2]), (W4[3], W5[3])]
        for a, (w4, w5) in enumerate(pairs):
            lo = _AGG0 + 128 * a
            inp[0:64, lo:lo + 64] = w4 + w5
            inp[0:64, lo + 64:lo + 128] = w5
        inp[0, _ONES0:_ONES0 + 128] = 1.0

        lhsT = inp[0:65, _LHS0:_LHS0 + 128]
        lh_hi = lhsT.astype(f16)
        lh_lo = (lhsT - lh_hi.astype(np.float32)).astype(f16)
        inpb = np.zeros((128, _BNF), f16)
        inpb[0:65, _BLH:_BLH + 128] = lh_hi
        inpb[0:65, _BLL:_BLL + 128] = lh_lo
        inpb[0:64, _BW3:_BW3 + 1024] = W3_16t
        in_maps.append({"inp": inp, "inpb": inpb})
    return in_maps


def _run(inputs, mask, nobj, coefs, bias, trace=False, **trace_kwargs):
    from concourse.bass_utils import run_bass_kernel_spmd

    in_maps = _host_pack(inputs, nobj, coefs, bias)
    nc = _get_nc()
    res = run_bass_kernel_spmd(nc, in_maps, list(range(B)), trace=trace,
                               **trace_kwargs)
    out = np.stack([res.results[i]["out"].reshape(N, N, S) for i in range(B)])
    m = np.asarray(mask, np.float32)
    if not np.all(m == 1.0):
        out = out * m  # mask is ones in the reference setup; host fallback
    return out, res


def kernel(inputs, mask, nobj, coefs, bias):
    out, _ = _run(inputs, mask, nobj, coefs, bias, trace=False)
    return out


if __name__ == "__main__":
    rng = np.random.default_rng(0)
    inputs = rng.standard_normal((B, N, C)).astype(np.float32)
    mask = np.ones((B, N, N, 1), np.float32)
    nobj = np.full((B,), 100.0, np.float32)
    coefs = (rng.standard_normal((C, S, 20)) * np.sqrt(2.0 / (C * 20))).astype(np.float32)
    bias = np.zeros((S,), np.float32)
    out = kernel(inputs, mask, nobj, coefs, bias)
    print("out", out.shape, out.dtype, float(np.abs(out).max()))


# revision 15
# speedup vs baseline: 1.0021x; 1.0021x over previous
"""Trainium2 Bass kernel for nn_Eq1to2 (segment_reduce / equivariant 1->2 layer).

Math (derived from the reference):
  out[n,i,j,s] = leaky_relu( A[n,i,s] + B[n,j,s] + G[n,s]
                             + (i==j) * (D[n,i,s] + Gd[n,s]) ) * mask
with
  A  = x @ W3                       (col term, i-dependent)
  B  = x @ W2                       (row term, j-dependent)
  D  = x @ W1                       (extra diagonal term)
  G  = sum_a agg_a @ W5_a + bias    (per-sample constant)
  Gd = sum_a agg_a @ W4_a           (per-sample diagonal constant)
where the 20 basis ops collapse to W1..W3 = sums of 4 coef slices each and
per-aggregation W4_a / W5_a; agg_a in {sum/49, sum/nobj, max, min} over N.

Sharding: pure data parallel, 1 batch sample per NeuronCore (B=8, 8 cores).

Device strategy per core (output tile [i=128 part, (j,s)=8192 free] fp32):
  - per 512-col chunk, 2 accumulating fp16 matmuls (fp16 rhs; fp16 hi/lo
    split of lhsT, error ~2^-11 of the rhs rounding only) with
    lhsT=[xT; ones] (K=65) and
    rhs=[W3 tiled 128x along j; flat(B + G + bias)] compute A + B + G + bias
    in PSUM (TensorE does both the partition and free broadcasts).
  - W3rep is materialized by SBUF->SBUF DMA with a stride-0 broadcast source.
  - ACT evicts PSUM->SBUF, DVE computes leaky via one fused
    scalar_tensor_tensor: out = (z * 0.01) max z.
  - bulk DMA the [128, 8192] tile to DRAM, then one small strided DMA
    overwrites the 128 diagonal (i==j) rows with the corrected
    leaky(A+B+D+G+Gd+bias) values computed exactly in fp32 (diagonal of
    [N,N,S] is a regular stride-(N+1)*S pattern in linear DRAM; HWDGE DMAs
    are FIFO per engine, plus explicit scheduler deps).
"""

import numpy as np

B, N, C, S = 8, 128, 64, 64
AVG_NOBJ = np.float32(49.0)
NEG = 0.01

# fp32 packed input column layout
_LHS0 = 0        # lhsT [65, 128]: rows 0:64 xT, row 64 ones (fp32, small mms)
_RHSS = 128      # rhs_small [65, 128]: cols 0:64 diag W, 64:128 B' W
_AGG0 = 256      # 3 x [64, 128] agg rhs (sum-combined, max, min)
_ONES0 = 640     # ones row at partition 0 [1, 128] (K=1 matmul lhsT)
_INF = 768       # total packed fp32 cols

# fp16 packed input column layout
_BLH = 0         # lhsT_hi [65, 128] (row 64 ones)
_BLL = 128       # lhsT_lo [65, 128] (row 64 zero)
_BW3 = 256       # W3 (fp16) tiled x32 [64, 2048]
_BNF = 2304

_CACHE = {}


def _build_nc():
    import concourse.bacc as bacc
    import concourse.bass as bass
    import concourse.mybir as mybir
    from concourse.tile import TileContext
    from concourse.tile_rust import add_dep_helper

    F32 = mybir.dt.float32
    FP16 = mybir.dt.float16
    Alu = mybir.AluOpType

    nc = bacc.Bacc("TRN2", debug=False, num_devices=8)
    inp_d = nc.dram_tensor("inp", [128, _INF], F32, kind="ExternalInput")
    inpb_d = nc.dram_tensor("inpb", [128, _BNF], FP16, kind="ExternalInput")
    out_d = nc.dram_tensor("out", [128, N * S], F32, kind="ExternalOutput")

    NB = 8           # big chunks
    CW = 1024        # chunk width (free elems)

    with TileContext(nc) as tc:
        with tc.tile_pool(name="main", bufs=1) as pool, \
             tc.tile_pool(name="tz", bufs=3) as tzpool, \
             tc.tile_pool(name="pz", bufs=3, space="PSUM") as pzpool, \
             tc.tile_pool(name="psm", bufs=1, space="PSUM") as psmpool:

            inp = pool.tile([128, _INF], F32)
            inpb = pool.tile([128, _BNF], FP16)
            rhs_t = []
            for c in range(NB // 2):
                rhs_c = pool.tile([65, 2048], FP16, tag=f"rhs{c}")
                rhs_t.append(rhs_c)
            outbuf = pool.tile([128, N * S], F32)
            aggs = pool.tile([64, 4], F32)
            ggrow = pool.tile([1, 64], F32)
            g16f = pool.tile([1, 64], F32)
            bp_hi = pool.tile([128, 64], FP16)
            dz = pool.tile([128, 64], F32)
            dleaky = pool.tile([128, 64], F32)

            nc.sync.dma_start(out=inp[:, :], in_=inp_d[:, :])
            nc.sync.dma_start(out=inpb[:, :], in_=inpb_d[:, :])

            lhsT = inp[0:65, _LHS0:_LHS0 + 128]
            ones_p0 = inp[0:1, _ONES0:_ONES0 + 128]
            xT = inp[0:64, _LHS0:_LHS0 + 128]
            lh_hi = inpb[0:65, _BLH:_BLH + 128]
            lh_lo = inpb[0:65, _BLL:_BLL + 128]
            w3h32 = inpb[0:64, _BW3:_BW3 + 2048]

            # W3rep rows 0:64 per rhs tile: copy the host-pretiled block
            # (small per-tile writer sets keep matmul deps minimal)
            for c in range(NB // 2):
                nc.gpsimd.dma_start(out=rhs_t[c][0:64, :], in_=w3h32)

            # aggregations over N (free dim of xT)
            nc.vector.tensor_reduce(out=aggs[:, 0:1], in_=xT,
                                    axis=mybir.AxisListType.X, op=Alu.add)
            nc.vector.tensor_reduce(out=aggs[:, 1:2], in_=xT,
                                    axis=mybir.AxisListType.X, op=Alu.max)
            nc.vector.tensor_reduce(out=aggs[:, 2:3], in_=xT,
                                    axis=mybir.AxisListType.X, op=Alu.min)

            # B' matmul: B' = x @ W2 + bias (ones row), then += G via a
            # K=1 matmul, so row 64 of rhs carries the full B+G+bias
            psum_sm = psmpool.tile([128, 128], F32)
            psum_bp = psum_sm[:, 64:128]
            psum_diag = psum_sm[:, 0:64]
            nc.tensor.matmul(psum_bp, lhsT,
                             inp[0:65, _RHSS + 64:_RHSS + 128],
                             start=True, stop=False)

            # [Gd + G | G] row via 3 accumulating M=1 matmuls
            # (host packs W4+W5 into the diag agg cols)
            psum_gg = psmpool.tile([1, 128], F32)
            for a in range(3):
                nc.tensor.matmul(psum_gg[0:1, :], aggs[:, a:a + 1],
                                 inp[0:64, _AGG0 + 128 * a:_AGG0 + 128 * (a + 1)],
                                 start=(a == 0), stop=(a == 2))
            nc.scalar.copy(ggrow[:, :], psum_gg[0:1, 0:64])
            nc.vector.tensor_copy(g16f[:, :], psum_gg[0:1, 64:128])
            nc.tensor.matmul(psum_bp, ones_p0, g16f[0:1, :],
                             start=False, stop=True)
            nc.scalar.copy(bp_hi[:, :], psum_bp)
            # per-tile flatten of the BG row into each rhs tile's row 64
            for c in range(NB // 2):
                nc.gpsimd.dma_start(out=rhs_t[c][64:65, :],
                                    in_=bp_hi[32 * c:32 * (c + 1), :])

            # diag matmul: diag_z = x @ (W1+W2+W3) + bias + (Gd+G)
            nc.tensor.matmul(psum_diag, lhsT, inp[0:65, _RHSS:_RHSS + 64],
                             start=True, stop=False)
            nc.tensor.matmul(psum_diag, ones_p0, ggrow[0:1, :],
                             start=False, stop=True)

            # diag path (exact fp32): leaky(A+B+D+G+Gd+bias)
            nc.scalar.copy(dz[:, :], psum_diag)
            nc.vector.scalar_tensor_tensor(out=dleaky[:, :], in0=dz[:, :],
                                           scalar=NEG, in1=dz[:, :],
                                           op0=Alu.mult, op1=Alu.max)

            # big chunks: 2x fp16 matmul (hh+lh) -> ACT evict -> DVE fused
            # leaky -> outbuf -> per-chunk bulk DMA -> per-chunk diagonal
            # overwrite on the same HWDGE ring (FIFO per engine)
            flat = out_d[:, :].rearrange("a b -> (a b)")
            bulks = []
            for c in range(NB):
                pz = pzpool.tile([128, CW], F32)
                for h2 in range(CW // 512):
                    o = pz[:, h2 * 512:(h2 + 1) * 512]
                    r = rhs_t[c // 2][0:65,
                                      (c % 2) * CW + h2 * 512:
                                      (c % 2) * CW + (h2 + 1) * 512]
                    nc.tensor.matmul(o, lh_hi, r, start=True, stop=False)
                    nc.tensor.matmul(o, lh_lo, r, start=False, stop=True)
                last = (c == NB - 1)
                for h in range(2 if last else 1):
                    w = CW // 2 if last else CW
                    sl = slice(c * CW + h * w, c * CW + (h + 1) * w)
                    t = tzpool.tile([128, CW], F32, tag="t")
                    nc.scalar.copy(t[:, 0:w], pz[:, h * w:(h + 1) * w])
                    nc.vector.scalar_tensor_tensor(
                        out=outbuf[:, sl], in0=t[:, 0:w], scalar=NEG,
                        in1=t[:, 0:w], op0=Alu.mult, op1=Alu.max)
                    eng = nc.sync if c % 2 == 0 else nc.scalar
                    bulk = eng.dma_start(out=out_d[:, sl], in_=outbuf[:, sl])
                    bulks.append(bulk)
                    if c == 5:
                        # diagonal rows 0:96 (chunks 0..5) in one DMA
                        dap = bass.AP(flat.tensor, flat.offset,
                                      [[(N + 1) * S, 96], [1, S]])
                        ddma = eng.dma_start(out=dap, in_=dleaky[0:96, :])
                        for b in bulks:
                            add_dep_helper(ddma.ins, b.ins,
                                           reason="diag after bulks 0-5")
                    elif c >= 6 and (not last or h == 1):
                        dap = bass.AP(flat.tensor,
                                      flat.offset + 16 * c * (N + 1) * S,
                                      [[(N + 1) * S, 16], [1, S]])
                        ddma = eng.dma_start(out=dap,
                                             in_=dleaky[16 * c:16 * (c + 1), :])
                        add_dep_helper(ddma.ins, bulk.ins,
                                       reason="diag overwrite after bulk")

    nc.compile()
    return nc


def _get_nc():
    if "nc" not in _CACHE:
        _CACHE["nc"] = _build_nc()
    return _CACHE["nc"]


def _host_pack(inputs, nobj, coefs, bias):
    import ml_dtypes

    x = np.asarray(inputs, np.float32)        # [B, N, C]
    nobj = np.asarray(nobj, np.float32)       # [B]
    c = np.asarray(coefs, np.float32)         # [C, S, 20]
    bias = np.asarray(bias, np.float32)       # [S]

    W1 = c[:, :, 0] + c[:, :, 5] + c[:, :, 10] + c[:, :, 15]
    W2 = c[:, :, 1] + c[:, :, 6] + c[:, :, 11] + c[:, :, 16]
    W3 = c[:, :, 2] + c[:, :, 7] + c[:, :, 12] + c[:, :, 17]
    W4 = [c[:, :, 3 + 5 * a] for a in range(4)]   # sum, mean, max, min
    W5 = [c[:, :, 4 + 5 * a] for a in range(4)]

    f16 = np.float16
    W3_16t = np.tile(W3.astype(f16), (1, 32))

    in_maps = []
    for n in range(B):
        inp = np.zeros((128, _INF), np.float32)
        inp[0:64, _LHS0:_LHS0 + 128] = x[n].T
        inp[64, _LHS0:_LHS0 + 128] = 1.0
        inp[0:64, _RHSS:_RHSS + 64] = W1 + W2 + W3
        inp[64, _RHSS:_RHSS + 64] = bias
        inp[0:64, _RHSS + 64:_RHSS + 128] = W2
        inp[64, _RHSS + 64:_RHSS + 128] = bias
        W4sm = W4[0] / AVG_NOBJ + W4[1] / nobj[n]
        W5sm = W5[0] / AVG_NOBJ + W5[1] / nobj[n]
        pairs = [(W4sm, W5sm), (W4[2], W5[2]), (W4[3], W5[3])]
        for a, (w4, w5) in enumerate(pairs):
            lo = _AGG0 + 128 * a
            inp[0:64, lo:lo + 64] = w4 + w5
            inp[0:64, lo + 64:lo + 128] = w5
        inp[0, _ONES0:_ONES0 + 128] = 1.0

        lhsT = inp[0:65, _LHS0:_LHS0 + 128]
        lh_hi = lhsT.astype(f16)
        lh_lo = (lhsT - lh_hi.astype(np.float32)).astype(f16)
        inpb = np.zeros((128, _BNF), f16)
        inpb[0:65, _BLH:_BLH + 128] = lh_hi
        inpb[0:65, _BLL:_BLL + 128] = lh_lo
        inpb[0:64, _BW3:_BW3 + 2048] = W3_16t
        in_maps.append({"inp": inp, "inpb": inpb})
    return in_maps


def _run(inputs, mask, nobj, coefs, bias, trace=False, **trace_kwargs):
    from concourse.bass_utils import run_bass_kernel_spmd

    in_maps = _host_pack(inputs, nobj, coefs, bias)
    nc = _get_nc()
    res = run_bass_kernel_spmd(nc, in_maps, list(range(B)), trace=trace,
                               **trace_kwargs)
    out = np.stack([res.results[i]["out"].reshape(N, N, S) for i in range(B)])
    m = np.asarray(mask, np.float32)
    if not np.all(m == 1.0):
        out = out * m  # mask is ones in the reference setup; host fallback
    return out, res


def kernel(inputs, mask, nobj, coefs, bias):
    out, _ = _run(inputs, mask, nobj, coefs, bias, trace=False)
    return out


if __name__ == "__main__":
    rng = np.random.default_rng(0)
    inputs = rng.standard_normal((B, N, C)).astype(np.float32)
    mask = np.ones((B, N, N, 1), np.float32)
    nobj = np.full((B,), 100.0, np.float32)
    coefs = (rng.standard_normal((C, S, 20)) * np.sqrt(2.0 / (C * 20))).astype(np.float32)
    bias = np.zeros((S,), np.float32)
    out = kernel(inputs, mask, nobj, coefs, bias)
    print("out", out.shape, out.dtype, float(np.abs(out).max()))
